# revision 14
# baseline (speedup 1.0000x reference)
"""BEV feature extractor (scatter-max -> 1x1 conv -> BN(train) -> ReLU) on 8 TRN2 cores.

Sharding: data-parallel over (batch, y-strip) -> 8 shards, BN stats all-reduced.

Device pipeline per core (all plain DMA + PE/DVE/ACT; indirect DMA only in the
small collision-fold step, using the canonical one-index-per-partition form):

  1. Host packs the shard: cells are grouped into 128-cell blocks; the occupied
     cells' "root" points of SLOT_BLKS consecutive blocks are packed into one
     128-row *slot*. r0 (DRAM input) holds root features in slot-major order.
     Colliding extra points are packed into fold batches of 128 with their
     target row index; a per-batch level schedule bounds collision depth.
  2. comb <- r0 (DRAM->DRAM copy). For each fold batch: per-channel indirect
     gather of the 128 root rows from r0, DVE elementwise max with each level's
     extras, per-channel indirect scatter into comb. comb = per-cell max.
  3. V[p,s] <- comb (slot-major) stays resident in SBUF. PE accumulates
     Sigma = sum_s V_s^T V_s and sv = sum_s V_s^T 1 (only occupied cells
     contribute; empty cells are zero rows). AllReduce(+) over 8 cores, then
     BN constants a = gamma/sqrt(var+eps), b = beta - mean*a are derived from
     mean = (W sv)/N, E[x^2] = diag(W Sigma W^T)/N  (empty cells contribute 0).
  4. Per slot: GT = V_s^T @ Sel_s (one matmul densifies the slot's cells into
     [c, cells] layout -- gather and transpose in one op; Sel is a host-built
     0/1 matrix), then feat = W^T_chunk @ GT, then ACT applies
     relu(feat*a + b) and the result streams to the output slab.
"""

import math
from dataclasses import dataclass

import numpy as np

import concourse.bass as bass
import concourse.tile as tile
from concourse import bacc, mybir
from concourse.bass_utils import run_bass_kernel_spmd

F32 = mybir.dt.float32
F32R = mybir.dt.float32r
I32 = mybir.dt.int32


@dataclass(frozen=True)
class Geo:
    B: int = 2
    H: int = 400
    W: int = 400
    C: int = 128            # input channels (= partition count)
    O: int = 256            # output channels (multiple of 128)
    NSTRIP: int = 4         # y-strips per batch; B*NSTRIP = 8 cores
    SLOT_BLKS: int = 2      # 128-cell blocks packed per 128-row slot
    NB: int = 21            # fold batches (128 colliding roots each)
    LVLS: tuple = (5, 2, 2) # per-batch fold depth; batches beyond get depth 1
    EPS: float = 1e-5
    SEL_DT: str = "float32"     # dtype of the selection matrices
    MM_DT: str = "float32"      # dtype tag for gather/conv matmuls (f32 or f32r)

    @property
    def ystrip(self):
        return self.H // self.NSTRIP

    @property
    def cells(self):
        return self.ystrip * self.W

    @property
    def ncores(self):
        return self.B * self.NSTRIP

    @property
    def slot_cells(self):
        return 128 * self.SLOT_BLKS

    @property
    def nslot(self):
        return math.ceil(self.cells / self.slot_cells)

    @property
    def nrows(self):                 # rows in r0/comb incl. 128 dump rows
        return self.nslot * 128 + 128

    @property
    def lvls(self):
        return tuple(self.LVLS) + (1,) * (self.NB - len(self.LVLS))

    @property
    def npair(self):                 # (batch, level) pairs
        return sum(self.lvls)

    @property
    def ncell_total(self):
        return self.B * self.H * self.W


GEO = Geo()


# --------------------------------------------------------------------------
# host-side shard prep
# --------------------------------------------------------------------------

def prep_shard(g: Geo, feats: np.ndarray, cell: np.ndarray) -> dict:
    """feats [n, C] f32, cell [n] int in [0, g.cells)."""
    C = g.C
    order = np.argsort(cell, kind="stable")
    cell_s = cell[order]
    feats_s = feats[order]
    uniq, seg_start, inverse, counts = np.unique(
        cell_s, return_index=True, return_inverse=True, return_counts=True
    )
    rank = np.arange(len(cell_s)) - seg_start[inverse]

    # --- slot packing: cell j -> slot j // slot_cells; occupied cells of a
    # slot occupy consecutive rows (cell order) within the slot's 128 rows.
    slot_of_uniq = uniq // g.slot_cells
    # row-within-slot: running index of occupied cells inside each slot
    row_in_slot = np.zeros(len(uniq), np.int64)
    occ_per_slot = np.zeros(g.nslot, np.int64)
    np.add.at(occ_per_slot, slot_of_uniq, 1)
    assert occ_per_slot.max(initial=0) <= 128, (
        f"slot overflow: {occ_per_slot.max()}"
    )
    first_of_slot = np.zeros(g.nslot, np.int64)
    first_of_slot[1:] = np.cumsum(occ_per_slot)[:-1]
    row_in_slot = np.arange(len(uniq)) - first_of_slot[slot_of_uniq]
    rowid = slot_of_uniq * 128 + row_in_slot          # row in r0/comb

    r0 = np.zeros((g.nrows, C), np.float32)
    m0 = rank == 0
    r0[rowid[inverse[m0]]] = feats_s[m0]

    # --- extras -> fold batches. Roots sorted by multiplicity desc so the
    # per-batch level schedule (lvls) covers the deepest collisions first.
    lvls = g.lvls
    exi = np.zeros((128, g.NB), np.int32)
    exi[:, :] = g.nslot * 128 + np.arange(128)[:, None]   # dump rows
    exf = np.zeros((128, g.npair, C), np.float32)
    pair_base = np.cumsum((0,) + lvls[:-1])
    has_extra = counts > 1
    ord_me = np.argsort(-counts[has_extra], kind="stable")
    me_uniq = np.flatnonzero(has_extra)[ord_me]           # uniq ids, mult desc
    nme = len(me_uniq)
    assert nme <= 128 * g.NB, f"fold batch capacity exceeded: {nme}"
    bi = np.arange(nme) // 128
    pi = np.arange(nme) % 128
    assert (counts[me_uniq] - 1 <= np.asarray(lvls)[bi]).all(), (
        "collision depth exceeds fold schedule"
    )
    exi[pi, bi] = rowid[me_uniq].astype(np.int32)
    pos_in_me = np.zeros(len(uniq), np.int64)
    pos_in_me[me_uniq] = np.arange(nme)
    for k in range(1, int(counts.max(initial=1))):
        mk = rank == k
        if not mk.any():
            continue
        pm = pos_in_me[inverse[mk]]            # batch position of the root
        exf[pm % 128, pair_base[pm // 128] + (k - 1)] = feats_s[mk]

    # --- selection matrices [nslot, 128, slot_cells]
    sel = np.zeros((g.nslot, 128, g.slot_cells), np.float32)
    sel[slot_of_uniq, row_in_slot, uniq % g.slot_cells] = 1.0
    return {"r0": r0, "exi": exi, "exf": exf, "sel": sel}


def prep_inputs(g: Geo, features, coordinates, conv_w, gamma, beta):
    feats = np.ascontiguousarray(features, np.float32)
    coords = np.asarray(coordinates)
    b, y, x = coords[:, 0], coords[:, 2], coords[:, 3]
    strip = y // g.ystrip
    wt = np.ascontiguousarray(conv_w.T, np.float32)                 # [C, O]
    gam = np.ascontiguousarray(
        np.asarray(gamma, np.float32).reshape(g.O // 128, 128).T)   # [128, O/128]
    bet = np.ascontiguousarray(
        np.asarray(beta, np.float32).reshape(g.O // 128, 128).T)
    in_maps = []
    for core in range(g.ncores):
        bb, st = divmod(core, g.NSTRIP)
        m = (b == bb) & (strip == st)
        cell = (y[m] - st * g.ystrip) * g.W + x[m]
        shard = prep_shard(g, feats[m], cell.astype(np.int64))
        shard.update({"wt": wt, "gamma": gam, "beta": bet})
        in_maps.append(shard)
    return in_maps


# --------------------------------------------------------------------------
# device program
# --------------------------------------------------------------------------

def build_program(g: Geo, debug: bool = False) -> bass.Bass:
    C, O = g.C, g.O
    OCH = O // 128
    NS = g.nslot
    SC = g.slot_cells
    lvls = g.lvls
    pair_base = [0]
    for l in lvls[:-1]:
        pair_base.append(pair_base[-1] + l)
    mmdt = F32 if g.MM_DT == "float32" else F32R

    nc = bacc.Bacc(num_devices=g.ncores)
    dbg_d = (
        nc.declare_dram_parameter("dbg", [128, 2 * (C + 1) + 8 * OCH], F32, True)
        if debug
        else None
    )
    r0_d = nc.declare_dram_parameter("r0", [g.nrows, C], F32, False)
    exi_d = nc.declare_dram_parameter("exi", [128, g.NB], I32, False)
    exf_d = nc.declare_dram_parameter("exf", [128, g.npair, C], F32, False)
    sel_d = nc.declare_dram_parameter("sel", [NS, 128, SC], F32, False)
    wt_d = nc.declare_dram_parameter("wt", [C, O], F32, False)
    gam_d = nc.declare_dram_parameter("gamma", [128, OCH], F32, False)
    bet_d = nc.declare_dram_parameter("beta", [128, OCH], F32, False)
    out_d = nc.declare_dram_parameter("out", [O, g.cells], F32, True)

    comb = nc.dram_tensor("comb", [g.nrows, C], F32)
    cc_in = nc.dram_tensor("cc_in", [C, C + 1], F32)
    cc_out = nc.dram_tensor("cc_out", [C, C + 1], F32, addr_space="Shared")

    with tile.TileContext(nc) as tc:
        with (
            tc.tile_pool(name="vstore", bufs=1) as vstore,
            tc.tile_pool(name="singles", bufs=1) as singles,
            tc.tile_pool(name="fold", bufs=2) as fold,
            tc.tile_pool(name="selp", bufs=3) as selp,
            tc.tile_pool(name="gtp", bufs=2) as gtpool,
            tc.tile_pool(name="osb", bufs=4) as opool,
            tc.tile_pool(name="pstat", bufs=1, space="PSUM") as pstat,
            tc.tile_pool(name="pgt", bufs=2, space="PSUM") as pgt,
            tc.tile_pool(name="pf", bufs=3, space="PSUM") as pf,
        ):
            # ---- small inputs
            ones = singles.tile([128, 1], F32)
            nc.vector.memset(ones[:], 1.0)
            wt_sb = singles.tile([C, O], F32)
            nc.sync.dma_start(out=wt_sb[:], in_=wt_d[:, :])
            gam_sb = singles.tile([128, OCH], F32)
            nc.sync.dma_start(out=gam_sb[:], in_=gam_d[:, :])
            bet_sb = singles.tile([128, OCH], F32)
            nc.sync.dma_start(out=bet_sb[:], in_=bet_d[:, :])
            exi_sb = singles.tile([128, g.NB], I32)
            nc.sync.dma_start(out=exi_sb[:], in_=exi_d[:, :])
            exf_sb = singles.tile([128, g.npair, C], F32)
            nc.sync.dma_start(out=exf_sb[:], in_=exf_d[:, :, :])

            # ---- comb <- r0 (plain copy), then fold collision batches
            nc.sync.dma_start(out=comb[:, :], in_=r0_d[:, :])
            for b in range(g.NB):
                gt = fold.tile([128, C], F32, tag="fold")
                nc.gpsimd.indirect_dma_start(
                    out=gt[:], out_offset=None, in_=r0_d[:, :],
                    in_offset=bass.IndirectOffsetOnAxis(
                        ap=exi_sb[:, b : b + 1], axis=0
                    ),
                )
                for l in range(lvls[b]):
                    nc.vector.tensor_tensor(
                        out=gt[:], in0=gt[:],
                        in1=exf_sb[:, pair_base[b] + l, :],
                        op=mybir.AluOpType.max,
                    )
                nc.gpsimd.indirect_dma_start(
                    out=comb[:, :],
                    out_offset=bass.IndirectOffsetOnAxis(
                        ap=exi_sb[:, b : b + 1], axis=0
                    ),
                    in_=gt[:], in_offset=None,
                )

            # ---- V tiles resident in SBUF (slot-major comb rows)
            comb3 = comb.ap().rearrange("(s p) c -> p s c", p=128)
            v_all = vstore.tile([128, NS, C], F32)
            LD = 40
            for s0 in range(0, NS, LD):
                s1 = min(NS, s0 + LD)
                nc.sync.dma_start(
                    out=v_all[:, s0:s1, :], in_=comb3[:, s0:s1, :]
                )

            # ---- Sigma / sv accumulation, AllReduce
            sig_ps = pstat.tile([128, C + 1], F32, space="PSUM")
            for s in range(NS):
                nc.tensor.matmul(
                    out=sig_ps[:, :C], lhsT=v_all[:, s, :], rhs=v_all[:, s, :],
                    start=(s == 0), stop=(s == NS - 1),
                )
            for s in range(NS):
                nc.tensor.matmul(
                    out=sig_ps[:, C : C + 1], lhsT=v_all[:, s, :], rhs=ones[:],
                    start=(s == 0), stop=(s == NS - 1),
                )
            sig_loc = singles.tile([128, C + 1], F32)
            nc.vector.tensor_copy(out=sig_loc[:], in_=sig_ps[:])
            nc.sync.dma_start(out=cc_in[:, :], in_=sig_loc[:])
            nc.gpsimd.collective_compute(
                "AllReduce",
                mybir.AluOpType.add,
                replica_groups=[list(range(g.ncores))],
                ins=[cc_in.ap().opt()],
                outs=[cc_out.ap().opt()],
            )
            sig_sb = singles.tile([128, C + 1], F32)
            nc.sync.dma_start(out=sig_sb[:], in_=cc_out[:, :])

            # ---- BN constants: a = gamma/sqrt(var+eps), b = beta - mean*a
            a_ps = pstat.tile([128, O], F32, space="PSUM")
            nc.tensor.matmul(
                out=a_ps[:], lhsT=sig_sb[:, :C], rhs=wt_sb[:],
                start=True, stop=True,
            )
            bsb = singles.tile([128, O], F32)
            nc.vector.tensor_tensor(
                out=bsb[:], in0=a_ps[:], in1=wt_sb[:], op=mybir.AluOpType.mult
            )
            red_ps = pstat.tile([128, 2 * OCH], F32, space="PSUM")
            for ch in range(OCH):
                nc.tensor.matmul(
                    out=red_ps[:, ch : ch + 1],
                    lhsT=bsb[:, ch * 128 : (ch + 1) * 128],
                    rhs=ones[:], start=True, stop=True,
                )
                nc.tensor.matmul(
                    out=red_ps[:, OCH + ch : OCH + ch + 1],
                    lhsT=wt_sb[:, ch * 128 : (ch + 1) * 128],
                    rhs=sig_sb[:, C : C + 1], start=True, stop=True,
                )
            inv_n = 1.0 / float(g.ncell_total)
            mom = singles.tile([128, 2 * OCH], F32)      # [ex2 | mean]
            nc.scalar.mul(out=mom[:], in_=red_ps[:], mul=inv_n)
            var_t = singles.tile([128, OCH], F32)
            nc.vector.tensor_tensor(
                out=var_t[:], in0=mom[:, OCH:], in1=mom[:, OCH:],
                op=mybir.AluOpType.mult,
            )
            nc.vector.tensor_tensor(
                out=var_t[:], in0=mom[:, :OCH], in1=var_t[:],
                op=mybir.AluOpType.subtract,
            )
            eps_t = singles.tile([128, 1], F32)
            nc.vector.memset(eps_t[:], float(g.EPS))
            rstd = singles.tile([128, OCH], F32)
            nc.scalar.activation(
                out=rstd[:], in_=var_t[:],
                func=mybir.ActivationFunctionType.Sqrt, bias=eps_t[:],
            )
            nc.vector.reciprocal(out=rstd[:], in_=rstd[:])
            a_t = singles.tile([128, OCH], F32)
            nc.vector.tensor_tensor(
                out=a_t[:], in0=gam_sb[:], in1=rstd[:], op=mybir.AluOpType.mult
            )
            b_t = singles.tile([128, OCH], F32)
            nc.vector.tensor_tensor(
                out=b_t[:], in0=mom[:, OCH:], in1=a_t[:], op=mybir.AluOpType.mult
            )
            nc.vector.tensor_tensor(
                out=b_t[:], in0=bet_sb[:], in1=b_t[:], op=mybir.AluOpType.subtract
            )
            if dbg_d is not None:
                nc.sync.dma_start(out=dbg_d[:, : C + 1], in_=sig_loc[:])
                nc.sync.dma_start(out=dbg_d[:, C + 1 : 2 * C + 2], in_=sig_sb[:])
                base = 2 * C + 2
                for t in [mom, var_t, rstd, a_t, b_t]:
                    w = t.shape[-1]
                    nc.sync.dma_start(out=dbg_d[:, base : base + w], in_=t[:])
                    base += w

            # ---- phase C: select+transpose, conv, BN+ReLU, store
            for s in range(NS):
                n_s = min(SC, g.cells - s * SC)
                sel_sb = selp.tile([128, SC], F32, tag="sel")
                nc.sync.dma_start(out=sel_sb[:, :n_s], in_=sel_d[s, :, :n_s])
                gt_ps = pgt.tile([128, SC], F32, space="PSUM")
                nc.tensor.matmul(
                    out=gt_ps[:, :n_s],
                    lhsT=v_all[:, s, :].bitcast(mmdt),
                    rhs=sel_sb[:, :n_s].bitcast(mmdt),
                    start=True, stop=True,
                )
                gt_sb = gtpool.tile([128, SC], F32, tag="gt")
                nc.vector.tensor_copy(out=gt_sb[:, :n_s], in_=gt_ps[:, :n_s])
                for ch in range(OCH):
                    fp = pf.tile([128, SC], F32, space="PSUM", tag="fp")
                    nc.tensor.matmul(
                        out=fp[:, :n_s],
                        lhsT=wt_sb[:, ch * 128 : (ch + 1) * 128].bitcast(mmdt),
                        rhs=gt_sb[:, :n_s].bitcast(mmdt),
                        start=True, stop=True,
                    )
                    ot = opool.tile([128, SC], F32, tag="osb")
                    nc.scalar.activation(
                        out=ot[:, :n_s], in_=fp[:, :n_s],
                        func=mybir.ActivationFunctionType.Relu,
                        scale=a_t[:, ch : ch + 1],
                        bias=b_t[:, ch : ch + 1],
                    )
                    nc.scalar.dma_start(
                        out=out_d[ch * 128 : (ch + 1) * 128, s * SC : s * SC + n_s],
                        in_=ot[:, :n_s],
                    )
    return nc


_PROGRAM_CACHE: dict = {}


def get_program(g: Geo) -> bass.Bass:
    if g not in _PROGRAM_CACHE:
        nc = build_program(g)
        # run_bass_via_pjrt serializes nc as-is; the Bacc lowering passes
        # (register allocation, 1-wait-per-instruction splitting) run in
        # finalize(), so it must happen before dispatch.
        nc.finalize()
        _PROGRAM_CACHE[g] = nc
    return _PROGRAM_CACHE[g]


def assemble_output(g: Geo, per_core: list) -> np.ndarray:
    out = np.empty((g.B, g.O, g.H, g.W), np.float32)
    for core in range(g.ncores):
        bb, st = divmod(core, g.NSTRIP)
        out[bb, :, st * g.ystrip : (st + 1) * g.ystrip, :] = per_core[
            core
        ].reshape(g.O, g.ystrip, g.W)
    return out


def kernel(features, coordinates, conv_w, gamma, beta):
    g = GEO
    in_maps = prep_inputs(g, features, coordinates, conv_w, gamma, beta)
    nc = get_program(g)
    res = run_bass_kernel_spmd(nc, in_maps, core_ids=list(range(g.ncores)))
    return assemble_output(g, [r["out"] for r in res.results])


# revision 19
# speedup vs baseline: 1.2401x; 1.2401x over previous
"""BEV feature extractor (scatter-max -> 1x1 conv -> BN(train) -> ReLU) on 8 TRN2 cores.

Sharding: data-parallel over (batch, y-strip) -> 8 shards, BN stats all-reduced.

Device pipeline per core (all plain DMA + PE/DVE/ACT; indirect DMA only in the
small collision-fold step, using the canonical one-index-per-partition form):

  1. Host packs the shard: cells are grouped into 128-cell blocks; the occupied
     cells' "root" points of SLOT_BLKS consecutive blocks are packed into one
     128-row *slot*. r0 (DRAM input) holds root features in slot-major order.
     Colliding extra points are packed into fold batches of 128 with their
     target row index; a per-batch level schedule bounds collision depth.
  2. comb <- r0 (DRAM->DRAM copy). For each fold batch: per-channel indirect
     gather of the 128 root rows from r0, DVE elementwise max with each level's
     extras, per-channel indirect scatter into comb. comb = per-cell max.
  3. V[p,s] <- comb (slot-major) stays resident in SBUF. PE accumulates
     Sigma = sum_s V_s^T V_s and sv = sum_s V_s^T 1 (only occupied cells
     contribute; empty cells are zero rows). AllReduce(+) over 8 cores, then
     BN constants a = gamma/sqrt(var+eps), b = beta - mean*a are derived from
     mean = (W sv)/N, E[x^2] = diag(W Sigma W^T)/N  (empty cells contribute 0).
  4. Per slot: GT = V_s^T @ Sel_s (one matmul densifies the slot's cells into
     [c, cells] layout -- gather and transpose in one op; Sel is a host-built
     0/1 matrix), then feat = W^T_chunk @ GT, then ACT applies
     relu(feat*a + b) and the result streams to the output slab.
"""

import math
from dataclasses import dataclass

import numpy as np

import concourse.bass as bass
import concourse.tile as tile
from concourse import bacc, mybir
from concourse.bass_utils import run_bass_kernel_spmd

F32 = mybir.dt.float32
F32R = mybir.dt.float32r
I32 = mybir.dt.int32


@dataclass(frozen=True)
class Geo:
    B: int = 2
    H: int = 400
    W: int = 400
    C: int = 128            # input channels (= partition count)
    O: int = 256            # output channels (multiple of 128)
    NSTRIP: int = 4         # y-strips per batch; B*NSTRIP = 8 cores
    SLOT_BLKS: int = 2      # 128-cell blocks packed per 128-row slot
    NB: int = 21            # fold batches (128 colliding roots each)
    LVLS: tuple = (5, 2, 2) # per-batch fold depth; batches beyond get depth 1
    EPS: float = 1e-5
    SEL_DT: str = "float32"     # dtype of the selection matrices
    MM_DT: str = "float32r"     # dtype tag for gather/conv matmuls (f32 or f32r)

    @property
    def ystrip(self):
        return self.H // self.NSTRIP

    @property
    def cells(self):
        return self.ystrip * self.W

    @property
    def ncores(self):
        return self.B * self.NSTRIP

    @property
    def slot_cells(self):
        return 128 * self.SLOT_BLKS

    @property
    def nslot(self):
        return math.ceil(self.cells / self.slot_cells)

    @property
    def nrows(self):                 # rows in r0/comb incl. 128 dump rows
        return self.nslot * 128 + 128

    @property
    def lvls(self):
        return tuple(self.LVLS) + (1,) * (self.NB - len(self.LVLS))

    @property
    def npair(self):                 # (batch, level) pairs
        return sum(self.lvls)

    @property
    def ncell_total(self):
        return self.B * self.H * self.W


GEO = Geo()


# --------------------------------------------------------------------------
# host-side shard prep
# --------------------------------------------------------------------------

def prep_shard(g: Geo, feats: np.ndarray, cell: np.ndarray) -> dict:
    """feats [n, C] f32, cell [n] int in [0, g.cells)."""
    C = g.C
    order = np.argsort(cell, kind="stable")
    cell_s = cell[order]
    feats_s = feats[order]
    uniq, seg_start, inverse, counts = np.unique(
        cell_s, return_index=True, return_inverse=True, return_counts=True
    )
    rank = np.arange(len(cell_s)) - seg_start[inverse]

    # --- slot packing: cell j -> slot j // slot_cells; occupied cells of a
    # slot occupy consecutive rows (cell order) within the slot's 128 rows.
    slot_of_uniq = uniq // g.slot_cells
    # row-within-slot: running index of occupied cells inside each slot
    row_in_slot = np.zeros(len(uniq), np.int64)
    occ_per_slot = np.zeros(g.nslot, np.int64)
    np.add.at(occ_per_slot, slot_of_uniq, 1)
    assert occ_per_slot.max(initial=0) <= 128, (
        f"slot overflow: {occ_per_slot.max()}"
    )
    first_of_slot = np.zeros(g.nslot, np.int64)
    first_of_slot[1:] = np.cumsum(occ_per_slot)[:-1]
    row_in_slot = np.arange(len(uniq)) - first_of_slot[slot_of_uniq]
    rowid = slot_of_uniq * 128 + row_in_slot          # row in r0/comb

    r0 = np.zeros((g.nrows, C), np.float32)
    m0 = rank == 0
    r0[rowid[inverse[m0]]] = feats_s[m0]

    # --- extras -> fold batches. Roots sorted by multiplicity desc so the
    # per-batch level schedule (lvls) covers the deepest collisions first.
    lvls = g.lvls
    exi = np.zeros((128, g.NB), np.int32)
    exi[:, :] = g.nslot * 128 + np.arange(128)[:, None]   # dump rows
    exf = np.zeros((128, g.npair, C), np.float32)
    pair_base = np.cumsum((0,) + lvls[:-1])
    has_extra = counts > 1
    ord_me = np.argsort(-counts[has_extra], kind="stable")
    me_uniq = np.flatnonzero(has_extra)[ord_me]           # uniq ids, mult desc
    nme = len(me_uniq)
    assert nme <= 128 * g.NB, f"fold batch capacity exceeded: {nme}"
    bi = np.arange(nme) // 128
    pi = np.arange(nme) % 128
    assert (counts[me_uniq] - 1 <= np.asarray(lvls)[bi]).all(), (
        "collision depth exceeds fold schedule"
    )
    exi[pi, bi] = rowid[me_uniq].astype(np.int32)
    pos_in_me = np.zeros(len(uniq), np.int64)
    pos_in_me[me_uniq] = np.arange(nme)
    for k in range(1, int(counts.max(initial=1))):
        mk = rank == k
        if not mk.any():
            continue
        pm = pos_in_me[inverse[mk]]            # batch position of the root
        exf[pm % 128, pair_base[pm // 128] + (k - 1)] = feats_s[mk]

    # --- selection matrices [nslot, 128, slot_cells]
    sel = np.zeros((g.nslot, 128, g.slot_cells), np.float32)
    sel[slot_of_uniq, row_in_slot, uniq % g.slot_cells] = 1.0
    return {"r0": r0, "exi": exi, "exf": exf, "sel": sel}


def prep_inputs(g: Geo, features, coordinates, conv_w, gamma, beta):
    feats = np.ascontiguousarray(features, np.float32)
    coords = np.asarray(coordinates)
    b, y, x = coords[:, 0], coords[:, 2], coords[:, 3]
    strip = y // g.ystrip
    wt = np.ascontiguousarray(conv_w.T, np.float32)                 # [C, O]
    gam = np.ascontiguousarray(
        np.asarray(gamma, np.float32).reshape(g.O // 128, 128).T)   # [128, O/128]
    bet = np.ascontiguousarray(
        np.asarray(beta, np.float32).reshape(g.O // 128, 128).T)
    in_maps = []
    for core in range(g.ncores):
        bb, st = divmod(core, g.NSTRIP)
        m = (b == bb) & (strip == st)
        cell = (y[m] - st * g.ystrip) * g.W + x[m]
        shard = prep_shard(g, feats[m], cell.astype(np.int64))
        shard.update({"wt": wt, "gamma": gam, "beta": bet})
        in_maps.append(shard)
    return in_maps


# --------------------------------------------------------------------------
# device program
# --------------------------------------------------------------------------

def build_program(g: Geo, debug: bool = False) -> bass.Bass:
    C, O = g.C, g.O
    OCH = O // 128
    NS = g.nslot
    SC = g.slot_cells
    lvls = g.lvls
    pair_base = [0]
    for l in lvls[:-1]:
        pair_base.append(pair_base[-1] + l)
    mmdt = F32 if g.MM_DT == "float32" else F32R

    nc = bacc.Bacc(num_devices=g.ncores)
    dbg_d = (
        nc.declare_dram_parameter("dbg", [128, 2 * (C + 1) + 8 * OCH], F32, True)
        if debug
        else None
    )
    r0_d = nc.declare_dram_parameter("r0", [g.nrows, C], F32, False)
    exi_d = nc.declare_dram_parameter("exi", [128, g.NB], I32, False)
    exf_d = nc.declare_dram_parameter("exf", [128, g.npair, C], F32, False)
    sel_d = nc.declare_dram_parameter("sel", [NS, 128, SC], mmdt, False)
    wt_d = nc.declare_dram_parameter("wt", [C, O], F32, False)
    gam_d = nc.declare_dram_parameter("gamma", [128, OCH], F32, False)
    bet_d = nc.declare_dram_parameter("beta", [128, OCH], F32, False)
    out_d = nc.declare_dram_parameter("out", [O, g.cells], F32, True)

    comb = nc.dram_tensor("comb", [g.nrows, C], F32)
    cc_in = nc.dram_tensor("cc_in", [C, C + 1], F32)
    cc_out = nc.dram_tensor("cc_out", [C, C + 1], F32, addr_space="Shared")

    with tile.TileContext(nc) as tc:
        with (
            tc.tile_pool(name="vstore", bufs=1) as vstore,
            tc.tile_pool(name="singles", bufs=1) as singles,
            tc.tile_pool(name="fold", bufs=2) as fold,
            tc.tile_pool(name="selp", bufs=3) as selp,
            tc.tile_pool(name="gtp", bufs=2) as gtpool,
            tc.tile_pool(name="osb", bufs=4) as opool,
            tc.tile_pool(name="pstat", bufs=1, space="PSUM") as pstat,
            tc.tile_pool(name="pgt", bufs=2, space="PSUM") as pgt,
            tc.tile_pool(name="pf", bufs=3, space="PSUM") as pf,
        ):
            # ---- small inputs
            ones = singles.tile([128, 1], F32)
            nc.vector.memset(ones[:], 1.0)
            wt_sb = singles.tile([C, O], F32)
            nc.sync.dma_start(out=wt_sb[:], in_=wt_d[:, :])
            wt_r = singles.tile([C, O], mmdt)
            if mmdt == F32:
                nc.sync.dma_start(out=wt_r[:], in_=wt_d[:, :])
            else:
                nc.gpsimd.dma_start(out=wt_r[:], in_=wt_d[:, :])
            gam_sb = singles.tile([128, OCH], F32)
            nc.sync.dma_start(out=gam_sb[:], in_=gam_d[:, :])
            bet_sb = singles.tile([128, OCH], F32)
            nc.sync.dma_start(out=bet_sb[:], in_=bet_d[:, :])
            exi_sb = singles.tile([128, g.NB], I32)
            nc.sync.dma_start(out=exi_sb[:], in_=exi_d[:, :])
            exf_sb = singles.tile([128, g.npair, C], F32)
            nc.sync.dma_start(out=exf_sb[:], in_=exf_d[:, :, :])

            # ---- comb <- r0 (plain copy), then fold collision batches
            nc.sync.dma_start(out=comb[:, :], in_=r0_d[:, :])
            for b in range(g.NB):
                gt = fold.tile([128, C], F32, tag="fold")
                nc.gpsimd.indirect_dma_start(
                    out=gt[:], out_offset=None, in_=r0_d[:, :],
                    in_offset=bass.IndirectOffsetOnAxis(
                        ap=exi_sb[:, b : b + 1], axis=0
                    ),
                )
                for l in range(lvls[b]):
                    nc.vector.tensor_tensor(
                        out=gt[:], in0=gt[:],
                        in1=exf_sb[:, pair_base[b] + l, :],
                        op=mybir.AluOpType.max,
                    )
                nc.gpsimd.indirect_dma_start(
                    out=comb[:, :],
                    out_offset=bass.IndirectOffsetOnAxis(
                        ap=exi_sb[:, b : b + 1], axis=0
                    ),
                    in_=gt[:], in_offset=None,
                )

            # ---- V tiles resident in SBUF (slot-major comb rows). Kept in
            # f32r (rounded once via the casting DMA) for the phase-C matmuls.
            comb3 = comb.ap().rearrange("(s p) c -> p s c", p=128)
            v_all = vstore.tile([128, NS, C], mmdt)
            LD = 40
            for s0 in range(0, NS, LD):
                s1 = min(NS, s0 + LD)
                if mmdt == F32:
                    nc.sync.dma_start(out=v_all[:, s0:s1, :], in_=comb3[:, s0:s1, :])
                else:
                    nc.gpsimd.dma_start(out=v_all[:, s0:s1, :], in_=comb3[:, s0:s1, :])

            # ---- Sigma / sv accumulation in full fp32 (streamed from comb),
            # with a fused ones-column: lhsT=V_s, rhs=[V_s | 1] -> [Sigma | sv].
            sig_ps = pstat.tile([128, C + 1], F32, space="PSUM")
            SIGLD = 16
            for s0 in range(0, NS, SIGLD):
                s1 = min(NS, s0 + SIGLD)
                vs = fold.tile([128, SIGLD, C + 1], F32, tag="sigv", bufs=3)
                nc.vector.memset(vs[:, :, C : C + 1], 1.0)
                nc.sync.dma_start(
                    out=vs[:, : s1 - s0, :C], in_=comb3[:, s0:s1, :]
                )
                for s in range(s0, s1):
                    nc.tensor.matmul(
                        out=sig_ps[:],
                        lhsT=vs[:, s - s0, :C],
                        rhs=vs[:, s - s0, :],
                        start=(s == 0), stop=(s == NS - 1),
                    )
            sig_loc = singles.tile([128, C + 1], F32)
            nc.vector.tensor_copy(out=sig_loc[:], in_=sig_ps[:])
            nc.sync.dma_start(out=cc_in[:, :], in_=sig_loc[:])
            nc.gpsimd.collective_compute(
                "AllReduce",
                mybir.AluOpType.add,
                replica_groups=[list(range(g.ncores))],
                ins=[cc_in.ap().opt()],
                outs=[cc_out.ap().opt()],
            )
            sig_sb = singles.tile([128, C + 1], F32)
            nc.sync.dma_start(out=sig_sb[:], in_=cc_out[:, :])

            # ---- BN constants: a = gamma/sqrt(var+eps), b = beta - mean*a
            a_ps = pstat.tile([128, O], F32, space="PSUM")
            nc.tensor.matmul(
                out=a_ps[:], lhsT=sig_sb[:, :C], rhs=wt_sb[:],
                start=True, stop=True,
            )
            bsb = singles.tile([128, O], F32)
            nc.vector.tensor_tensor(
                out=bsb[:], in0=a_ps[:], in1=wt_sb[:], op=mybir.AluOpType.mult
            )
            red_ps = pstat.tile([128, 2 * OCH], F32, space="PSUM")
            for ch in range(OCH):
                nc.tensor.matmul(
                    out=red_ps[:, ch : ch + 1],
                    lhsT=bsb[:, ch * 128 : (ch + 1) * 128],
                    rhs=ones[:], start=True, stop=True,
                )
                nc.tensor.matmul(
                    out=red_ps[:, OCH + ch : OCH + ch + 1],
                    lhsT=wt_sb[:, ch * 128 : (ch + 1) * 128],
                    rhs=sig_sb[:, C : C + 1], start=True, stop=True,
                )
            inv_n = 1.0 / float(g.ncell_total)
            mom = singles.tile([128, 2 * OCH], F32)      # [ex2 | mean]
            nc.scalar.mul(out=mom[:], in_=red_ps[:], mul=inv_n)
            var_t = singles.tile([128, OCH], F32)
            nc.vector.tensor_tensor(
                out=var_t[:], in0=mom[:, OCH:], in1=mom[:, OCH:],
                op=mybir.AluOpType.mult,
            )
            nc.vector.tensor_tensor(
                out=var_t[:], in0=mom[:, :OCH], in1=var_t[:],
                op=mybir.AluOpType.subtract,
            )
            eps_t = singles.tile([128, 1], F32)
            nc.vector.memset(eps_t[:], float(g.EPS))
            rstd = singles.tile([128, OCH], F32)
            nc.scalar.activation(
                out=rstd[:], in_=var_t[:],
                func=mybir.ActivationFunctionType.Sqrt, bias=eps_t[:],
            )
            nc.vector.reciprocal(out=rstd[:], in_=rstd[:])
            a_t = singles.tile([128, OCH], F32)
            nc.vector.tensor_tensor(
                out=a_t[:], in0=gam_sb[:], in1=rstd[:], op=mybir.AluOpType.mult
            )
            b_t = singles.tile([128, OCH], F32)
            nc.vector.tensor_tensor(
                out=b_t[:], in0=mom[:, OCH:], in1=a_t[:], op=mybir.AluOpType.mult
            )
            nc.vector.tensor_tensor(
                out=b_t[:], in0=bet_sb[:], in1=b_t[:], op=mybir.AluOpType.subtract
            )
            if dbg_d is not None:
                nc.sync.dma_start(out=dbg_d[:, : C + 1], in_=sig_loc[:])
                nc.sync.dma_start(out=dbg_d[:, C + 1 : 2 * C + 2], in_=sig_sb[:])
                base = 2 * C + 2
                for t in [mom, var_t, rstd, a_t, b_t]:
                    w = t.shape[-1]
                    nc.sync.dma_start(out=dbg_d[:, base : base + w], in_=t[:])
                    base += w

            # ---- phase C: select+transpose, conv, BN+ReLU, store.
            # Output DMAs are batched over slot pairs and alternate between
            # the two HWDGE rings (SP / ACT) to spread sequencer residency.
            ot_cur = [None] * OCH
            for s in range(NS):
                n_s = min(SC, g.cells - s * SC)
                pair0 = s % 2 == 0
                sel_sb = selp.tile([128, SC], mmdt, tag="sel")
                nc.sync.dma_start(out=sel_sb[:, :n_s], in_=sel_d[s, :, :n_s])
                gt_ps = pgt.tile([128, SC], F32, space="PSUM")
                nc.tensor.matmul(
                    out=gt_ps[:, :n_s],
                    lhsT=v_all[:, s, :],
                    rhs=sel_sb[:, :n_s],
                    start=True, stop=True,
                )
                gt_sb = gtpool.tile([128, SC], mmdt, tag="gt")
                nc.vector.tensor_copy(out=gt_sb[:, :n_s], in_=gt_ps[:, :n_s])
                for ch in range(OCH):
                    fp = pf.tile([128, SC], F32, space="PSUM", tag="fp")
                    nc.tensor.matmul(
                        out=fp[:, :n_s],
                        lhsT=wt_r[:, ch * 128 : (ch + 1) * 128],
                        rhs=gt_sb[:, :n_s],
                        start=True, stop=True,
                    )
                    if pair0:
                        ot_cur[ch] = opool.tile(
                            [128, 2 * SC], F32, tag=f"osb{ch}", name=f"ot{ch}"
                        )
                    ot = ot_cur[ch]
                    off = 0 if pair0 else SC
                    nc.scalar.activation(
                        out=ot[:, off : off + n_s], in_=fp[:, :n_s],
                        func=mybir.ActivationFunctionType.Relu,
                        scale=a_t[:, ch : ch + 1],
                        bias=b_t[:, ch : ch + 1],
                    )
                    if (not pair0) or s == NS - 1:
                        w = off + n_s
                        base = (s - (0 if pair0 else 1)) * SC
                        eng = nc.sync if (s // 2) % 2 == 0 else nc.scalar
                        eng.dma_start(
                            out=out_d[
                                ch * 128 : (ch + 1) * 128, base : base + w
                            ],
                            in_=ot[:, :w],
                        )
    return nc


_PROGRAM_CACHE: dict = {}


def get_program(g: Geo) -> bass.Bass:
    if g not in _PROGRAM_CACHE:
        nc = build_program(g)
        # run_bass_via_pjrt serializes nc as-is; the Bacc lowering passes
        # (register allocation, 1-wait-per-instruction splitting) run in
        # finalize(), so it must happen before dispatch.
        nc.finalize()
        _PROGRAM_CACHE[g] = nc
    return _PROGRAM_CACHE[g]


def assemble_output(g: Geo, per_core: list) -> np.ndarray:
    out = np.empty((g.B, g.O, g.H, g.W), np.float32)
    for core in range(g.ncores):
        bb, st = divmod(core, g.NSTRIP)
        out[bb, :, st * g.ystrip : (st + 1) * g.ystrip, :] = per_core[
            core
        ].reshape(g.O, g.ystrip, g.W)
    return out


def kernel(features, coordinates, conv_w, gamma, beta):
    g = GEO
    in_maps = prep_inputs(g, features, coordinates, conv_w, gamma, beta)
    nc = get_program(g)
    res = run_bass_kernel_spmd(nc, in_maps, core_ids=list(range(g.ncores)))
    return assemble_output(g, [r["out"] for r in res.results])
